# revision 101
# baseline (speedup 1.0000x reference)
"""Causal single-head attention on 8 trn2 NeuronCores.

Sharding: core c handles batch c//2 and half the query rows of that batch
(4 blocks of 256 rows picked so causal work balances). Uniform device
program; per-core data (host gather/scatter + causal masks) selects rows.

Algorithm (v3) — projections folded away AND all matmuls in fp8 e4m3
"3-slab" residual form with DoubleRow perf mode:
  every operand v is split hi = e4m3(v), lo = e4m3(v - hi); a product
  v*w keeps vh*wh + vl*wh + vh*wl (drops lo*lo, ~1e-3 rel).  DoubleRow
  contracts TWO 128-row K-slabs per PE instruction at 0.5 cyc/row, so the
  3 slab-products over a K-tile pair cost 0.75x the bf16 equivalent with
  better-than-bf16 accuracy (measured 1.9e-3 vs bf16's 3.3e-3).

  host:  A' = 64 (Wq^T Wk), Wv' = 32 Wv^T, both split hi/lo; x split
         hi/lo in three layouts (x^T keys, x^T own-query cols, x rows).
  dev:   T' = A'^T x_q^T                       (phase 1, [d2, i])
         S'^T[j,i] = x^T.T T'  (+ causal mask) per 256-query slot
         p32 = exp(S'/2048 + ln 16)  f32; ph = e4m3(p32), pl = resid
         U'^T[d,i] = sum_j x_row-slabs @ p-slabs   (16x unnorm. probs)
         l16[i] = ones^T (ph+pl) = 16 l           (PE matmul)
         uth = e4m3(U'/32), utl = resid
         out[i,o] = (ut-slabs^T @ Wv'-slabs) / l16   (= U Wv^T / l)
"""

import sys

try:
    import concourse  # noqa: F401
except ImportError:
    sys.path.insert(0, "/opt/trn_rl_repo")

import math
from contextlib import ExitStack

import ml_dtypes
import numpy as np

import concourse.bass as bass  # noqa: F401
from concourse import bacc
import concourse.mybir as mybir
import concourse.tile as tile
from concourse.bass_utils import run_bass_kernel_spmd

B, N, D = 4, 2048, 1024
NQ = 1024            # query rows owned per core
NCORES = 8
TLO = (14, 10, 6, 2)   # 256-wide j-tile trips per slot (uniform program)
THI = (16, 12, 8, 4)    # total trips incl. 2 extra 128-wide (B-half) tiles
# each slot pairs two 128-row query blocks (A = cols 0-127, B = 128-255)
# with different causal depths; assignment balances causal work across h
SLOTPAIRS = (((12, 15), (8, 11), (4, 7), (0, 3)),
             ((13, 14), (9, 10), (5, 6), (1, 2)))
IB = 256             # query block width
P = 128
F8 = mybir.dt.float8e4
BF = mybir.dt.bfloat16
F32 = mybir.dt.float32
NPF8 = ml_dtypes.float8_e4m3
NPBF = ml_dtypes.bfloat16
DR = mybir.MatmulPerfMode.DoubleRow
EXP_SCALE = 1.0 / 2048.0        # 1/(32*64): A pre-scaled by 64
EXP_BIAS = math.log(16.0)       # probs' = 16 p  (l cancels the 16)
UT_SCALE = 1.0 / 32.0           # uth = U'/32; Wv' = 32 Wv^T cancels

LAST_RESULT = None
LAST_IN_MAPS = None
_CACHED_NC = None


def _qrows(h):
    rows = []
    for qa, qb in SLOTPAIRS[h]:
        rows.append(np.arange(128 * qa, 128 * qa + 128))
        rows.append(np.arange(128 * qb, 128 * qb + 128))
    return np.concatenate(rows)


def _build_masks(h):
    """0/1 causal masks: m256 [4, 2, 128, 256] for the last two 256-wide
    j-tiles per slot; m128 [4, 2, 128, 128] for the two B-half-only tiles."""
    m256 = np.zeros((4, 2, P, IB), np.float32)
    m128 = np.zeros((4, 2, P, P), np.float32)
    jp = np.arange(P)[:, None]
    for s, (qa, qb) in enumerate(SLOTPAIRS[h]):
        qrow = np.concatenate([128 * qa + np.arange(P),
                               128 * qb + np.arange(P)])[None, :]
        for k in range(2):
            jt = TLO[s] - 2 + k
            m256[s, k] = (jt * P + jp <= qrow).astype(np.float32)
            jt = TLO[s] + k
            m128[s, k] = (jt * P + jp <= 128 * qb +
                          np.arange(P)[None, :]).astype(np.float32)
    return m256, m128


def _pairs(m, npair):
    """[npair*256, C] -> [128, 2*npair, C]: row 128*(2p+k)+dd -> [dd, 2p+k]."""
    C = m.shape[1]
    return np.ascontiguousarray(
        m.reshape(npair, 2, P, C).transpose(2, 0, 1, 3).reshape(P, 2 * npair, C))


def _split_pairs(m, npair):
    """f32 [npair*256, C] -> (hi, lo) e4m3 [128, 2*npair, C]."""
    m = np.asarray(m, np.float32)
    hi = m.astype(NPF8)
    lo = (m - hi.astype(np.float32)).astype(NPF8)
    return _pairs(hi, npair), _pairs(lo, npair)


def _mm3(nc, ps, wh, wl, ih, il, start, stop):
    """3-slab DoubleRow product group fragment: wh*ih + wl*ih + wh*il."""
    nc.tensor.matmul(ps, lhsT=wh, rhs=ih, start=start, stop=False,
                     perf_mode=DR)
    nc.tensor.matmul(ps, lhsT=wl, rhs=ih, start=False, stop=False,
                     perf_mode=DR)
    nc.tensor.matmul(ps, lhsT=wh, rhs=il, start=False, stop=stop,
                     perf_mode=DR)


def _build_body(nc, tc, ctx, dram, r):
    (ah_d, al_d, xqh_d, xql_d, xth_d, xtl_d, xnh_d, xnl_d,
     wvh_d, wvl_d, mask_d, maskn_d, out_d) = dram
    Exp = mybir.ActivationFunctionType.Exp
    mult = mybir.AluOpType.mult
    subtract = mybir.AluOpType.subtract

    pool_xt = ctx.enter_context(tc.tile_pool(name=f"xt{r}", bufs=2))
    pool_xn = ctx.enter_context(tc.tile_pool(name=f"xn{r}", bufs=1))
    pool_wv = ctx.enter_context(tc.tile_pool(name=f"wv{r}", bufs=1))
    pool_tt = ctx.enter_context(tc.tile_pool(name=f"tt{r}", bufs=8))
    pool_mask = ctx.enter_context(tc.tile_pool(name=f"mask{r}", bufs=1))
    pool_out = ctx.enter_context(tc.tile_pool(name=f"outb{r}", bufs=5))
    pool_one = ctx.enter_context(tc.tile_pool(name=f"one{r}", bufs=1))

    maskt = pool_mask.tile([P, 8, IB], F8, tag="mask", name=f"mask{r}")
    masknt = pool_mask.tile([P, 8, P], F8, tag="maskn", name=f"maskn{r}")
    # xt split into column halves (j-tiles 0-7 / 8-15) for earlier first use
    xthts = [pool_xt.tile([P, 8, N // 2], F8, tag="xth", name=f"xth{r}_{c}")
             for c in range(2)]
    xtlts = [pool_xt.tile([P, 8, N // 2], F8, tag="xtl", name=f"xtl{r}_{c}")
             for c in range(2)]
    xnht = pool_xn.tile([P, 16, D], F8, tag="xnh", name=f"xnh{r}")
    xnlt = pool_xn.tile([P, 16, D], F8, tag="xnl", name=f"xnl{r}")
    wvht = pool_wv.tile([P, 8, D], F8, tag="wvh", name=f"wvh{r}")
    wvlt = pool_wv.tile([P, 8, D], F8, tag="wvl", name=f"wvl{r}")
    ones = pool_one.tile([P, 2, 1], F8, tag="one", name=f"ones{r}")
    nc.vector.memset(ones, 1.0)
    ebias = pool_one.tile([P, 1], F32, tag="ebias", name=f"ebias{r}")
    nc.vector.memset(ebias, EXP_BIAS)

    # ---- phase 1: T'^T[d2, i] = sum_d1 A'[d1, d2] x_q^T[d1, i] ----
    # Contraction split into lo half (d1 pairs 0-1) and hi half (pairs 2-3)
    # accumulated in separate PSUM groups combined on DVE, so the PE can
    # start on the first-arriving DMA chunks.
    tth = [[pool_tt.tile([P, 2, 512], F8, tag="tth", name=f"tth{r}_{h}_{t}")
            for t in range(4)] for h in range(2)]
    ttl = [[pool_tt.tile([P, 2, 512], F8, tag="ttl", name=f"ttl{r}_{h}_{t}")
            for t in range(4)] for h in range(2)]
    with (
        tc.tile_pool(name=f"a{r}", bufs=4) as pool_a,
        tc.tile_pool(name=f"xq{r}", bufs=8) as pool_xq,
        tc.tile_pool(name=f"ps1{r}", bufs=5, space="PSUM") as ps1,
    ):
        # chunked loads ordered by first use: pairs 0-1 (the "A" part of the
        # contraction) before pairs 2-3 (the "B" part). a on scalar queue,
        # xq on sync queue so issue overlaps.
        ahs, als = [None] * 4, [None] * 4
        xqhs = [[None] * 2 for _ in range(4)]
        xqls = [[None] * 2 for _ in range(4)]

        def load_a(p4):
            t = pool_a.tile([P, 2, D], F8, tag="ah", name=f"ah{r}_{p4}")
            nc.scalar.dma_start(out=t, in_=ah_d[:, 2 * p4:2 * p4 + 2, :])
            ahs[p4] = t
            t = pool_a.tile([P, 2, D], F8, tag="al", name=f"al{r}_{p4}")
            nc.scalar.dma_start(out=t, in_=al_d[:, 2 * p4:2 * p4 + 2, :])
            als[p4] = t

        def load_xq(p4, half):
            c = slice(half * 512, half * 512 + 512)
            t = pool_xq.tile([P, 2, 512], F8, tag="xqh",
                             name=f"xqh{r}_{p4}_{half}")
            nc.sync.dma_start(out=t, in_=xqh_d[:, 2 * p4:2 * p4 + 2, c])
            xqhs[p4][half] = t
            t = pool_xq.tile([P, 2, 512], F8, tag="xql",
                             name=f"xql{r}_{p4}_{half}")
            nc.sync.dma_start(out=t, in_=xql_d[:, 2 * p4:2 * p4 + 2, c])
            xqls[p4][half] = t

        t = pool_xq.tile([P, 2, 512], F8, tag="xqh", name=f"xqh{r}_0_0")
        nc.sync.dma_start(out=t, in_=xqh_d[:, 0:2, 0:512])
        xqhs[0][0] = t
        load_a(0)
        t = pool_xq.tile([P, 2, 512], F8, tag="xql", name=f"xql{r}_0_0")
        nc.sync.dma_start(out=t, in_=xql_d[:, 0:2, 0:512])
        xqls[0][0] = t
        load_xq(1, 0); load_a(1)
        load_xq(2, 0); load_a(2)
        load_xq(3, 0); load_a(3)
        load_xq(0, 1); load_xq(1, 1)
        load_xq(2, 1); load_xq(3, 1)
        # bulk loads for later phases (sync queue, behind xq)
        H = N // 2
        nc.sync.dma_start(out=xthts[0], in_=xth_d[:, :, 0:H])
        nc.sync.dma_start(out=xtlts[0], in_=xtl_d[:, :, 0:H])
        nc.sync.dma_start(out=xthts[1], in_=xth_d[:, :, H:N])
        nc.sync.dma_start(out=xtlts[1], in_=xtl_d[:, :, H:N])
        nc.sync.dma_start(out=maskt, in_=mask_d[:, :, :])
        nc.sync.dma_start(out=masknt, in_=maskn_d[:, :, :])
        nc.sync.dma_start(out=xnht, in_=xnh_d[:, :, :])
        nc.sync.dma_start(out=xnlt, in_=xnl_d[:, :, :])

        # One PSUM group per output, contraction emitted in two stages:
        # pairs (0,1) ("A" instrs, group start) for up to 6 staged groups,
        # then pairs (2,3) ("B" instrs, group stop) as banks recycle.
        outs_order = [(d2, half) for half in range(2) for d2 in range(8)]
        groups = {}

        def p1_part(g, p4s, start, stop):
            d2, half = outs_order[g]
            if g not in groups:
                groups[g] = ps1.tile([P, 512], F32, tag="ps1",
                                     name=f"ps1{r}_{g}")
            ps = groups[g]
            for i, p4 in enumerate(p4s):
                _mm3(nc, ps,
                     ahs[p4][:, 0:2, d2 * P:(d2 + 1) * P],
                     als[p4][:, 0:2, d2 * P:(d2 + 1) * P],
                     xqhs[p4][half][:, 0:2, 0:512],
                     xqls[p4][half][:, 0:2, 0:512],
                     start=(start and i == 0),
                     stop=(stop and i == len(p4s) - 1))

        def p1_close(g):
            p1_part(g, (2, 3), start=False, stop=True)
            ps = groups.pop(g)
            d2, half = outs_order[g]
            th = tth[half][d2 // 2][:, d2 % 2:d2 % 2 + 1, :]
            tl = ttl[half][d2 // 2][:, d2 % 2:d2 % 2 + 1, :]
            nc.scalar.copy(th, ps)
            nc.vector.tensor_sub(tl, ps, th)

        # interleave p0 across staged groups first so the PE can run on the
        # earliest-arriving chunks alone
        for g in range(5):
            p1_part(g, (0,), start=True, stop=False)
        for g in range(5):
            p1_part(g, (1,), start=False, stop=False)
        for g in range(5, 16):
            p1_close(g - 5)
            p1_part(g, (0, 1), start=True, stop=False)
        for g in range(11, 16):
            p1_close(g)

    # ---- phase 2: attention: all scores passes, then all value passes ----
    with (
        tc.tile_pool(name=f"p32{r}", bufs=12) as pool_p32,
        tc.tile_pool(name=f"pm{r}", bufs=6) as pool_pm,
        tc.tile_pool(name=f"ph{r}", bufs=16) as pool_ph,
        tc.tile_pool(name=f"pl{r}", bufs=16) as pool_pl,
        tc.tile_pool(name=f"ut{r}", bufs=8) as pool_ut,
        tc.tile_pool(name=f"lr{r}", bufs=8) as pool_lr,
    ):
        slot_probs = {}
        slot_us = {}

        def out_pass(s, ps_f):
            for _ in out_chunks(s, ps_f):
                pass

        def score_jt(s, jt, ps_s):
            # jt < TLO[s]: full 256-wide tile; jt in {TLO, TLO+1}: 128-wide
            # tile covering only the B-half (cols 128-255) of the slot
            narrow = jt >= TLO[s]
            w = P if narrow else IB
            co = IB - w
            phs, pls = slot_probs.setdefault(s, ([], []))
            pss = ps_s.tile([P, w], F32, tag="pss",
                            name=f"pss{r}_{s}_{jt}")
            ci = (s % 2) * IB + co
            for dp in range(4):
                _mm3(nc, pss,
                     xthts[jt // 8][:, 2 * dp:2 * dp + 2,
                                    (jt % 8) * P:(jt % 8 + 1) * P],
                     xtlts[jt // 8][:, 2 * dp:2 * dp + 2,
                                    (jt % 8) * P:(jt % 8 + 1) * P],
                     tth[s // 2][dp][:, 0:2, ci:ci + w],
                     ttl[s // 2][dp][:, 0:2, ci:ci + w],
                     start=(dp == 0), stop=(dp == 3))
            p32 = pool_p32.tile([P, w], F32, tag="p32",
                                name=f"p32{r}_{s}_{jt}")
            nc.scalar.activation(p32, pss, Exp, scale=EXP_SCALE, bias=ebias)
            if narrow:
                k = jt - TLO[s]
                mk = masknt[:, s * 2 + k:s * 2 + k + 1, :]
            else:
                k = jt - (TLO[s] - 2)
                mk = (maskt[:, s * 2 + k:s * 2 + k + 1, :] if k >= 0
                      else None)
            if mk is not None:
                pm = pool_pm.tile([P, w], F32, tag="pm",
                                  name=f"pm{r}_{s}_{jt}")
                nc.gpsimd.tensor_mul(pm, p32, mk)
                srcv = pm
            else:
                srcv = p32
            if narrow:
                if jt == TLO[s]:
                    phs.append(pool_pm.tile([P, 2, P], F8, tag="phn",
                                            name=f"phn{r}_{s}"))
                    pls.append(pool_pm.tile([P, 2, P], F8, tag="pln",
                                            name=f"pln{r}_{s}"))
                kk = jt - TLO[s]
            else:
                if jt % 2 == 0:
                    phs.append(pool_ph.tile([P, 2, IB], F8, tag="ph",
                                            name=f"ph{r}_{s}_{jt // 2}"))
                    pls.append(pool_pl.tile([P, 2, IB], F8, tag="pl",
                                            name=f"pl{r}_{s}_{jt // 2}"))
                kk = jt % 2
            nc.vector.tensor_copy(phs[-1][:, kk:kk + 1, :], srcv)
            nc.vector.tensor_sub(pls[-1][:, kk:kk + 1, :], srcv,
                                 phs[-1][:, kk:kk + 1, :])

        def value_dt(s, dt, ps_u, ps_l):
            nprs = TLO[s] // 2
            phs, pls = slot_probs[s]
            psl2, uths, utls = slot_us.setdefault(
                s, (ps_l.tile([P, 2], F32, tag="l", name=f"psl{r}_{s}"),
                    [], []))
            psu = ps_u.tile([P, IB], F32, tag="u", name=f"psu{r}_{s}_{dt}")
            for t in range(nprs):
                _mm3(nc, psu,
                     xnht[:, 2 * t:2 * t + 2, dt * P:(dt + 1) * P],
                     xnlt[:, 2 * t:2 * t + 2, dt * P:(dt + 1) * P],
                     phs[t][:, 0:2, :], pls[t][:, 0:2, :],
                     start=(t == 0), stop=False)
                if dt < 4:
                    # l16: ones^T (ph+pl); one group, chunk per psl column
                    src = phs[t] if dt < 2 else pls[t]
                    ch = dt % 2
                    nc.tensor.matmul(
                        psl2[:, ch:ch + 1],
                        lhsT=src[:, 0:2, ch * P:(ch + 1) * P],
                        rhs=ones[:, 0:2, 0:1],
                        start=(t == 0 and dt == 0), stop=False,
                        perf_mode=DR)
            # narrow pair (j-tiles TLO, TLO+1): B-half columns only
            _mm3(nc, psu[:, P:IB],
                 xnht[:, TLO[s]:TLO[s] + 2, dt * P:(dt + 1) * P],
                 xnlt[:, TLO[s]:TLO[s] + 2, dt * P:(dt + 1) * P],
                 phs[nprs][:, 0:2, :], pls[nprs][:, 0:2, :],
                 start=False, stop=True)
            if dt in (1, 3):
                # narrow l contributions land in the B (hi) chunk
                src = phs[nprs] if dt == 1 else pls[nprs]
                nc.tensor.matmul(
                    psl2[:, 1:2], lhsT=src[:, 0:2, 0:P],
                    rhs=ones[:, 0:2, 0:1],
                    start=False, stop=(dt == 3), perf_mode=DR)
            if dt % 2 == 0:
                uths.append(pool_ut.tile([P, 2, IB], F8, tag="uth",
                                         name=f"uth{r}_{s}_{dt // 2}"))
                utls.append(pool_ut.tile([P, 2, IB], F8, tag="utl",
                                         name=f"utl{r}_{s}_{dt // 2}"))
            kk = dt % 2
            if s == 0:
                # s3-interleaved: keep ACT exp-only (in-order hazard)
                nc.vector.tensor_scalar_mul(uths[-1][:, kk:kk + 1, :], psu,
                                            UT_SCALE)
            else:
                nc.scalar.mul(uths[-1][:, kk:kk + 1, :], psu, UT_SCALE)
            nc.vector.scalar_tensor_tensor(
                utls[-1][:, kk:kk + 1, :], psu, UT_SCALE,
                uths[-1][:, kk:kk + 1, :], mult, subtract)

        def out_chunks(s, ps_f):
            # out[i, o] = (sum_d ut-slabs Wv'-slabs) / l16
            psl2, uths, utls = slot_us.pop(s)
            slot_probs.pop(s)
            for half in range(2):
                rt = pool_lr.tile([P, 1], F32, tag="lr",
                                  name=f"lrec{r}_{s}_{half}")
                nc.vector.reciprocal(rt, psl2[:, half:half + 1])
                r0 = s * IB + half * P
                # the very last output chunk is emitted as two 256-col psum
                # groups so the final obh+DMA chain is half as deep
                chunks = [(0, 512), (512, 512)]
                if s == 2 and half == 1:
                    chunks = [(0, 512), (512, 256), (768, 256)]
                for c0, w in chunks:
                    psf = ps_f.tile([P, w], F32, tag="f",
                                    name=f"psf{r}_{s}_{half}_{c0}")
                    for dp in range(4):
                        _mm3(nc, psf,
                             uths[dp][:, 0:2, half * P:(half + 1) * P],
                             utls[dp][:, 0:2, half * P:(half + 1) * P],
                             wvht[:, 2 * dp:2 * dp + 2, c0:c0 + w],
                             wvlt[:, 2 * dp:2 * dp + 2, c0:c0 + w],
                             start=(dp == 0), stop=(dp == 3))
                    obh = pool_out.tile([P, w], F32, tag="obh",
                                        name=f"obh{r}_{s}_{half}_{c0}")
                    nc.vector.tensor_scalar_mul(obh, psf, rt)
                    nc.sync.dma_start(out=out_d[r0:r0 + P, c0:c0 + w],
                                      in_=obh)
                    yield

        # Scores first (needs only xt + tt), values after (needs the later
        # xn/wv DMAs); all attention PSUM pools open together (2+3+2+1 = 8
        # banks) so value banks are fresh, and value(0)'s U dt-groups are
        # interleaved into scores(3)'s j-tiles to relax every chain.
        with (
            # ps_s declared last so it lands on the two banks phase 1 never
            # touched (ps1 has 6 bufs) - no bank-handover wait at scores start
            tc.tile_pool(name=f"ps_u{r}", bufs=3, space="PSUM") as ps_u,
            tc.tile_pool(name=f"ps_f{r}", bufs=2, space="PSUM") as ps_f,
            tc.tile_pool(name=f"ps_l{r}", bufs=1, space="PSUM") as ps_l,
            tc.tile_pool(name=f"ps_s{r}", bufs=2, space="PSUM") as ps_s,
        ):
            for jt in range(THI[0]):
                score_jt(0, jt, ps_s)
            for jt in range(THI[1]):
                score_jt(1, jt, ps_s)
            for jt in range(THI[2]):
                score_jt(2, jt, ps_s)
                if jt % 2 == 1:
                    value_dt(0, jt // 2, ps_u, ps_l)
            # wv issues from the scalar queue here: ACT's exec-queue depth is
            # 0, so its SEQ can only run ~4 instructions ahead of the exps —
            # these transfers launch mid-scores, after the xt/xn/out0 deps
            nc.scalar.dma_start(out=wvht, in_=wvh_d[:, :, :])
            nc.scalar.dma_start(out=wvlt, in_=wvl_d[:, :, :])
            for jt in range(THI[3]):
                score_jt(3, jt, ps_s)
                if jt % 2 == 1:
                    value_dt(0, 4 + jt // 2, ps_u, ps_l)
            for dt in range(6, 8):
                value_dt(0, dt, ps_u, ps_l)
            # each out pass hosts the next value slot's U groups between its
            # psf chunks: both psum-recycle chains get slack, and the psl
            # bank hand-off (freed at the host's recips) stays legal
            og = out_chunks(0, ps_f)
            next(og)
            for dt in range(8):
                value_dt(1, dt, ps_u, ps_l)
                if dt % 2 == 1:
                    next(og, None)
            for _ in og:
                pass
            og = out_chunks(1, ps_f)
            next(og)
            for dt in range(8):
                value_dt(3, dt, ps_u, ps_l)
                if dt % 2 == 1:
                    next(og, None)
            for _ in og:
                pass
            og = out_chunks(3, ps_f)
            next(og)
            for dt in range(8):
                value_dt(2, dt, ps_u, ps_l)
                if dt % 2 == 1:
                    next(og, None)
            for _ in og:
                pass
            out_pass(2, ps_f)


def _build_nc(reps=1):
    nc = bacc.Bacc(None, target_bir_lowering=False)

    ah_d = nc.declare_dram_parameter("ah", [P, 8, D], F8, isOutput=False)
    al_d = nc.declare_dram_parameter("al", [P, 8, D], F8, isOutput=False)
    xqh_d = nc.declare_dram_parameter("xqh", [P, 8, NQ], F8, isOutput=False)
    xql_d = nc.declare_dram_parameter("xql", [P, 8, NQ], F8, isOutput=False)
    xth_d = nc.declare_dram_parameter("xth", [P, 8, N], F8, isOutput=False)
    xtl_d = nc.declare_dram_parameter("xtl", [P, 8, N], F8, isOutput=False)
    xnh_d = nc.declare_dram_parameter("xnh", [P, 16, D], F8, isOutput=False)
    xnl_d = nc.declare_dram_parameter("xnl", [P, 16, D], F8, isOutput=False)
    wvh_d = nc.declare_dram_parameter("wvh", [P, 8, D], F8, isOutput=False)
    wvl_d = nc.declare_dram_parameter("wvl", [P, 8, D], F8, isOutput=False)
    mask_d = nc.declare_dram_parameter("masks", [P, 8, IB], F8,
                                       isOutput=False)
    maskn_d = nc.declare_dram_parameter("maskn", [P, 8, P], F8,
                                        isOutput=False)
    out_d = nc.declare_dram_parameter("out_p", [NQ, D], F32, isOutput=True)
    dram = (ah_d, al_d, xqh_d, xql_d, xth_d, xtl_d, xnh_d, xnl_d,
            wvh_d, wvl_d, mask_d, maskn_d, out_d)

    with tile.TileContext(nc) as tc:
        for rep in range(reps):
            with ExitStack() as ctx:
                _build_body(nc, tc, ctx, dram, rep)
    nc.finalize()
    return nc


def _make_in_maps(x, W_q, W_k, W_v):
    wq = np.asarray(W_q, np.float32)
    wk = np.asarray(W_k, np.float32)
    wv = np.asarray(W_v, np.float32)
    ah, al = _split_pairs(64.0 * (wq.T @ wk), 4)          # [d1, d2]
    wvh, wvl = _split_pairs(32.0 * wv.T, 4)               # [d, o]
    masks, masksn = [], []
    for h in range(2):
        m256, m128 = _build_masks(h)
        m256 = m256.transpose(2, 0, 1, 3).reshape(P, 8, IB)
        m128 = m128.transpose(2, 0, 1, 3).reshape(P, 8, P)
        masks.append(np.ascontiguousarray(m256).astype(NPF8))
        masksn.append(np.ascontiguousarray(m128).astype(NPF8))
    qrows = [_qrows(0), _qrows(1)]
    in_maps = []
    for c in range(NCORES):
        b, h = c // 2, c % 2
        xb = np.asarray(x[b], np.float32)
        xb_t = xb.T  # [D, N]
        xth, xtl = _split_pairs(xb_t, 4)
        xqh, xql = _split_pairs(xb_t[:, qrows[h]], 4)
        xnh, xnl = _split_pairs(xb, 8)
        in_maps.append({
            "ah": ah, "al": al, "xqh": xqh, "xql": xql,
            "xth": xth, "xtl": xtl, "xnh": xnh, "xnl": xnl,
            "wvh": wvh, "wvl": wvl, "masks": masks[h],
            "maskn": masksn[h],
        })
    return in_maps


def kernel(x, W_q, W_k, W_v):
    global _CACHED_NC, LAST_RESULT, LAST_IN_MAPS
    x = np.asarray(x, dtype=np.float32)
    if _CACHED_NC is None:
        _CACHED_NC = _build_nc()
    nc = _CACHED_NC

    in_maps = _make_in_maps(x, W_q, W_k, W_v)
    LAST_IN_MAPS = in_maps
    try:
        res = run_bass_kernel_spmd(nc, in_maps, list(range(NCORES)))
    except Exception:
        # transient NRT_EXEC_UNIT_UNRECOVERABLE wedges clear on retry
        import time as _time
        _time.sleep(5)
        res = run_bass_kernel_spmd(nc, in_maps, list(range(NCORES)))
    LAST_RESULT = res

    qrows = [_qrows(0), _qrows(1)]
    out = np.empty((B, N, D), np.float32)
    for c in range(NCORES):
        b, h = c // 2, c % 2
        out[b, qrows[h], :] = res.results[c]["out_p"]
    return out
